# revision 8
# baseline (speedup 1.0000x reference)
"""Causal self-attention (B=4, T=2048, C=1024, H=16) on 8 TRN2 NeuronCores.

Sharding: tensor-parallel pairs. Core c handles batch b = c//2 and head-half
j = c%2 (8 of the 16 heads). Each core computes the QKV projection for its
heads, causal attention, and the out-projection contracted over its half of
the features, producing a partial output. The pair-sum (the "all-reduce after
out_proj" of the tensor-parallel scheme) happens at unshard time on the host.

Matmuls run in float32r (TF32-like rounding of fp32 at full PE rate) except
the probs@V matmul, which runs in bf16 with 128-column weight windows (FWL).
Softmax runs in fp32: 1/sqrt(D) folded into the q projection, causal masking
via additive -1e30 before exp, and the denominator comes free from a
ones-column appended to V so the AV matmul accumulates sum(exp) in PSUM.

q/k/v stay resident in SBUF between the projection and attention phases (no
DRAM round-trip). Every core executes the identical program (SPMD); only the
input shards differ.
"""
import numpy as np
from contextlib import ExitStack

import concourse.bass as bass
from concourse import bacc
import concourse.mybir as mybir
import concourse.tile as tile
from concourse.bass_utils import run_bass_kernel_spmd

B, T, C, H, D = 4, 2048, 1024, 16, 64
NCORES = 8
HPC = H // 2          # heads per core
F = HPC * D           # 512 features per core (per q/k/v)
KI = C // 128         # 8 contraction tiles over C
NT = T // 512         # 4 token chunks
F32 = mybir.dt.float32
F32R = mybir.dt.float32r
BF16 = mybir.dt.bfloat16

_NC_CACHE = None


def _build():
    nc = bacc.Bacc("TRN2", target_bir_lowering=False, debug=False)
    # raw fp32 data declared as f32r: the PE rounds on read identically to an
    # explicit rounding pass (verified bit-exact on HW)
    xt = nc.dram_tensor("xt", [C, T], F32R, kind="ExternalInput").ap()
    wqkvt = nc.dram_tensor("wqkvt", [C, 3 * F], F32R, kind="ExternalInput").ap()
    woutt = nc.dram_tensor("woutt", [F, C], F32R, kind="ExternalInput").ap()
    out = nc.dram_tensor("out", [C, T], F32, kind="ExternalOutput").ap()

    with ExitStack() as ctx:
        tc = ctx.enter_context(tile.TileContext(nc))

        # SBUF-resident q/k/v for the whole kernel:
        #   qT[m][128f, T] (pre-scaled by 1/8), kT[m][128f, T] — feature-major
        #   vt[tm][128tk, 583] bf16 — token-major, 8 head-groups of 65 cols
        #   (64 v features + ones col), tail-padded so every 128-col FWL
        #   weight window stays in bounds; pad/ones cols only ever feed
        #   psum partitions >= 65 which are never read.
        qk = ctx.enter_context(tc.tile_pool(name="qk", bufs=1))
        vp = ctx.enter_context(tc.tile_pool(name="vp", bufs=1))
        qts = [qk.tile([128, T], F32R, tag=f"q{m}", name=f"q{m}") for m in range(4)]
        kts = [qk.tile([128, T], F32R, tag=f"k{m}", name=f"k{m}") for m in range(4)]
        vts = [vp.tile([128, 583], BF16, tag=f"v{tm}", name=f"v{tm}")
               for tm in range(T // 128)]

        # ---- Phase A: QKV projection, streamed over 512-token chunks ----
        with tc.tile_pool(name="a_w", bufs=1) as a_w, \
             tc.tile_pool(name="a_x", bufs=2) as a_x, \
             tc.tile_pool(name="a_ps", bufs=4, space="PSUM") as a_ps:
            wq = []
            for ki in range(KI):
                t = a_w.tile([128, 3 * F], F32R, tag=f"w{ki}", name=f"w{ki}")
                nc.sync.dma_start(out=t[:], in_=wqkvt[ki * 128:(ki + 1) * 128, :])
                wq.append(t)
            for tm in range(T // 128):
                nc.gpsimd.memset(vts[tm][:], 1.0)
            for n in range(NT):
                xc = []
                for ki in range(KI):
                    t = a_x.tile([128, 512], F32R, tag=f"xc{ki}", name=f"xc{ki}")
                    nc.sync.dma_start(
                        out=t[:], in_=xt[ki * 128:(ki + 1) * 128, n * 512:(n + 1) * 512])
                    xc.append(t)
                # q,k feature-major
                for m in range(8):      # m<4: q feature tiles, else k
                    pt = a_ps.tile([128, 512], F32, tag="pt", name="pt")
                    for ki in range(KI):
                        nc.tensor.matmul(pt[:], wq[ki][:, m * 128:(m + 1) * 128],
                                         xc[ki][:],
                                         start=(ki == 0), stop=(ki == KI - 1))
                    dst = (qts[m] if m < 4 else kts[m - 4])[:, n * 512:(n + 1) * 512]
                    if m < 4:
                        nc.vector.tensor_scalar_mul(dst, pt[:], 0.125)
                    else:
                        nc.vector.tensor_copy(dst, pt[:])
                # v token-major into the resident vt tiles
                for tmi in range(4):
                    tm = n * 4 + tmi
                    pt = a_ps.tile([128, 512], F32, tag="pt", name="pt")
                    for ki in range(KI):
                        nc.tensor.matmul(pt[:], xc[ki][:, tmi * 128:(tmi + 1) * 128],
                                         wq[ki][:, 2 * F:3 * F],
                                         start=(ki == 0), stop=(ki == KI - 1))
                    vdst = vts[tm][:, 0:520].rearrange("p (h c) -> p h c", c=65)
                    nc.vector.tensor_copy(
                        vdst[:, :, 0:64], pt[:].rearrange("p (h c) -> p h c", c=64))

        # ---- Phase B: attention + Phase C: out-projection ----
        with tc.tile_pool(name="const", bufs=1) as constp, \
             tc.tile_pool(name="wo", bufs=1) as wop, \
             tc.tile_pool(name="yp", bufs=2) as yp, \
             tc.tile_pool(name="bp", bufs=6) as bp, \
             tc.tile_pool(name="pbp", bufs=16) as pbp, \
             tc.tile_pool(name="co", bufs=2) as cop, \
             tc.tile_pool(name="bps", bufs=2, space="PSUM") as bps, \
             tc.tile_pool(name="yps", bufs=1, space="PSUM") as yps, \
             tc.tile_pool(name="cps", bufs=2, space="PSUM") as cps:
            # diagonal-chunk causal masks: mask[d][i, j] = 0 if j >= i + d*128
            # else -1e30   (scoresT orientation: partition i = key, free j = query)
            masks = []
            for dlt in range(4):
                mt = constp.tile([128, 2, 512], F32, tag=f"m{dlt}", name=f"m{dlt}")
                nc.gpsimd.memset(mt[:], 0.0)
                nc.gpsimd.affine_select(
                    out=mt[:], in_=mt[:],
                    compare_op=mybir.AluOpType.is_ge, fill=-1e30,
                    base=-dlt * 128, pattern=[[0, 2], [1, 512]],
                    channel_multiplier=-1)
                masks.append(mt)
            wo = []
            for ki in range(F // 128):
                t = wop.tile([128, C], F32R, tag=f"wo{ki}", name=f"wo{ki}")
                nc.sync.dma_start(out=t[:], in_=woutt[ki * 128:(ki + 1) * 128, :])
                wo.append(t)

            for qc in range(NT):
                n_kt = qc * 4 + 4
                yts = [yp.tile([128, 512], F32R, tag=f"y{i}", name=f"y{i}")
                       for i in range(4)]
                for hp in range(HPC // 2):       # head pairs (2*hp, 2*hp+1)
                    qpair = qts[hp][:, qc * 512:(qc + 1) * 512]
                    pyA = yps.tile([128, 512], F32, tag="pyA", name="pyA")
                    pyB = yps.tile([128, 512], F32, tag="pyB", name="pyB")
                    pbs = []
                    for kt in range(n_kt):
                        ksl = kts[hp][:, kt * 128:(kt + 1) * 128]
                        # causal trim: cols < lo are fully masked; keep N>=256
                        # (f32r matmul needs moving dim >= 256 for full rate)
                        lo = min(max((kt - qc * 4) * 128, 0), 256)
                        ps = bps.tile([128, 2, 512], F32, tag="ps", name="ps")
                        nc.tensor.matmul(ps[:, 0, lo:512], ksl[0:64, :],
                                         qpair[0:64, lo:512],
                                         start=True, stop=True, tile_position=(0, 0))
                        nc.tensor.matmul(ps[:, 1, lo:512], ksl[64:128, :],
                                         qpair[64:128, lo:512],
                                         start=True, stop=True, tile_position=(64, 0))
                        if kt >= qc * 4:
                            mk = masks[kt - qc * 4]
                            nc.vector.tensor_add(ps[:, :, lo:512], ps[:, :, lo:512],
                                                 mk[:, :, lo:512])
                        pb = pbp.tile([128, 2, 512], BF16, tag="pb", name="pb")
                        nc.scalar.activation(pb[:, :, lo:512], ps[:, :, lo:512],
                                             mybir.ActivationFunctionType.Exp)
                        pbs.append((kt, lo, pb))
                    for kt, lo, pb in pbs:
                        a0 = 2 * hp * 65
                        nc.tensor.matmul(pyA[:, lo:512], vts[kt][:, a0:a0 + 128],
                                         pb[:, 0, lo:512],
                                         start=(kt == 0), stop=(kt == n_kt - 1))
                    for kt, lo, pb in pbs:
                        a0 = 2 * hp * 65 + 65
                        nc.tensor.matmul(pyB[:, lo:512], vts[kt][:, a0:a0 + 128],
                                         pb[:, 1, lo:512],
                                         start=(kt == 0), stop=(kt == n_kt - 1))
                    for hh, py in ((0, pyA), (1, pyB)):
                        h = 2 * hp + hh
                        s1 = bp.tile([1, 512], F32, tag="s1", name="s1")
                        nc.vector.tensor_copy(s1[:], py[64:65, :])
                        r = bp.tile([1, 512], F32, tag="r", name="r")
                        nc.vector.reciprocal_approx_fast(out=r[:], in_=s1[:])
                        rb = bp.tile([64, 512], F32, tag="rb", name="rb")
                        nc.gpsimd.partition_broadcast(rb[:], r[:])
                        half = (h % 2) * 64
                        nc.vector.tensor_mul(yts[h // 2][half:half + 64, :],
                                             py[0:64, :], rb[:])
                # out-projection for this token chunk
                for m in range(8):
                    po = cps.tile([128, 512], F32, tag="po", name="po")
                    for ki in range(F // 128):
                        nc.tensor.matmul(po[:], wo[ki][:, m * 128:(m + 1) * 128],
                                         yts[ki][:],
                                         start=(ki == 0), stop=(ki == F // 128 - 1))
                    oo = cop.tile([128, 512], F32, tag="oo", name="oo")
                    nc.vector.tensor_copy(oo[:], po[:])
                    nc.sync.dma_start(
                        out=out[m * 128:(m + 1) * 128, qc * 512:(qc + 1) * 512],
                        in_=oo[:])
    nc.finalize()
    return nc


def _get_nc():
    global _NC_CACHE
    if _NC_CACHE is None:
        _NC_CACHE = _build()
    return _NC_CACHE


def kernel(x, w_qkv, w_out):
    x = np.ascontiguousarray(np.asarray(x), dtype=np.float32)
    w_qkv = np.asarray(w_qkv, dtype=np.float32)
    w_out = np.asarray(w_out, dtype=np.float32)
    nc = _get_nc()

    in_maps = []
    for c in range(NCORES):
        b, j = divmod(c, 2)
        rows = np.r_[j * F:(j + 1) * F,
                     C + j * F:C + (j + 1) * F,
                     2 * C + j * F:2 * C + (j + 1) * F]
        in_maps.append({
            "xt": np.ascontiguousarray(x[b].T),
            "wqkvt": np.ascontiguousarray(w_qkv[rows, :].T),
            "woutt": np.ascontiguousarray(w_out[:, j * F:(j + 1) * F].T),
        })

    res = run_bass_kernel_spmd(nc, in_maps, core_ids=list(range(NCORES)))
    y = np.empty((B, T, C), np.float32)
    for b in range(B):
        y[b] = (res.results[2 * b]["out"] + res.results[2 * b + 1]["out"]).T
    return y


# revision 10
# speedup vs baseline: 1.0403x; 1.0403x over previous
"""Causal self-attention (B=4, T=2048, C=1024, H=16) on 8 TRN2 NeuronCores.

Sharding: tensor-parallel pairs. Core c handles batch b = c//2 and head-half
j = c%2 (8 of the 16 heads). Each core computes the QKV projection for its
heads, causal attention, and the out-projection contracted over its half of
the features, producing a partial output. The pair-sum (the "all-reduce after
out_proj" of the tensor-parallel scheme) happens at unshard time on the host.

Matmuls run in float32r (TF32-like rounding of fp32 at full PE rate) except
the probs@V matmul, which runs in bf16 with 128-column weight windows (FWL).
Softmax runs in fp32: 1/sqrt(D) folded into the q projection, causal masking
via additive -1e30 before exp, and the denominator comes free from a
ones-column appended to V so the AV matmul accumulates sum(exp) in PSUM.

q/k/v stay resident in SBUF between the projection and attention phases (no
DRAM round-trip). Every core executes the identical program (SPMD); only the
input shards differ.
"""
import numpy as np
from contextlib import ExitStack

import concourse.bass as bass
from concourse import bacc
import concourse.mybir as mybir
import concourse.tile as tile
from concourse.bass_utils import run_bass_kernel_spmd

B, T, C, H, D = 4, 2048, 1024, 16, 64
NCORES = 8
HPC = H // 2          # heads per core
F = HPC * D           # 512 features per core (per q/k/v)
KI = C // 128         # 8 contraction tiles over C
NT = T // 512         # 4 token chunks
F32 = mybir.dt.float32
F32R = mybir.dt.float32r
BF16 = mybir.dt.bfloat16

_NC_CACHE = None


def _build():
    nc = bacc.Bacc("TRN2", target_bir_lowering=False, debug=False)
    # raw fp32 data declared as f32r: the PE rounds on read identically to an
    # explicit rounding pass (verified bit-exact on HW)
    xt = nc.dram_tensor("xt", [C, T], F32R, kind="ExternalInput").ap()
    wqkvt = nc.dram_tensor("wqkvt", [C, 3 * F], F32R, kind="ExternalInput").ap()
    woutt = nc.dram_tensor("woutt", [F, C], F32R, kind="ExternalInput").ap()
    out = nc.dram_tensor("out", [C, T], F32, kind="ExternalOutput").ap()

    with ExitStack() as ctx:
        tc = ctx.enter_context(tile.TileContext(nc))

        # SBUF-resident q/k/v for the whole kernel:
        #   qT[m][128f, T] (pre-scaled by 1/8), kT[m][128f, T] — feature-major
        #   vt[tm][128tk, 583] bf16 — token-major, 8 head-groups of 65 cols
        #   (64 v features + ones col), tail-padded so every 128-col FWL
        #   weight window stays in bounds; pad/ones cols only ever feed
        #   psum partitions >= 65 which are never read.
        qk = ctx.enter_context(tc.tile_pool(name="qk", bufs=1))
        vp = ctx.enter_context(tc.tile_pool(name="vp", bufs=1))
        qts = [qk.tile([128, T], F32R, tag=f"q{m}", name=f"q{m}") for m in range(4)]
        kts = [qk.tile([128, T], F32R, tag=f"k{m}", name=f"k{m}") for m in range(4)]
        vts = [vp.tile([128, 583], BF16, tag=f"v{tm}", name=f"v{tm}")
               for tm in range(T // 128)]

        # ---- Phase A: QKV projection, streamed over 512-token chunks ----
        with tc.tile_pool(name="a_w", bufs=1) as a_w, \
             tc.tile_pool(name="a_x", bufs=2) as a_x, \
             tc.tile_pool(name="a_ps", bufs=2, space="PSUM") as a_ps:
            wq = []
            for ki in range(KI):
                t = a_w.tile([128, 3 * F], F32R, tag=f"w{ki}", name=f"w{ki}")
                wq.append(t)
            for tm in range(T // 128):
                nc.gpsimd.memset(vts[tm][:], 1.0)
            for n in range(NT):
                xc = []
                for ki in range(KI):
                    if n == 0:
                        # interleave weight/input loads in consumption order so
                        # ki-major accumulation starts after the first pair lands
                        nc.sync.dma_start(out=wq[ki][:],
                                          in_=wqkvt[ki * 128:(ki + 1) * 128, :])
                    t = a_x.tile([128, 512], F32R, tag=f"xc{ki}", name=f"xc{ki}")
                    nc.sync.dma_start(
                        out=t[:], in_=xt[ki * 128:(ki + 1) * 128, n * 512:(n + 1) * 512])
                    xc.append(t)
                # q,k feature-major — ki-major over groups of 4 psum banks
                for mg in (0, 4):
                    pts = [a_ps.tile([128, 512], F32, tag=f"pt{i}", name=f"pt{i}")
                           for i in range(4)]
                    for ki in range(KI):
                        for i in range(4):
                            m = mg + i
                            nc.tensor.matmul(pts[i][:], wq[ki][:, m * 128:(m + 1) * 128],
                                             xc[ki][:],
                                             start=(ki == 0), stop=(ki == KI - 1))
                    for i in range(4):
                        m = mg + i
                        dst = (qts[m] if m < 4 else kts[m - 4])[:, n * 512:(n + 1) * 512]
                        if m < 4:
                            nc.vector.tensor_scalar_mul(dst, pts[i][:], 0.125)
                        else:
                            nc.vector.tensor_copy(dst, pts[i][:])
                # v token-major — ki-major over the chunk's 4 token tiles
                pts = [a_ps.tile([128, 512], F32, tag=f"pt{i}", name=f"pt{i}")
                       for i in range(4)]
                for ki in range(KI):
                    for tmi in range(4):
                        nc.tensor.matmul(pts[tmi][:],
                                         xc[ki][:, tmi * 128:(tmi + 1) * 128],
                                         wq[ki][:, 2 * F:3 * F],
                                         start=(ki == 0), stop=(ki == KI - 1))
                for tmi in range(4):
                    tm = n * 4 + tmi
                    vdst = vts[tm][:, 0:520].rearrange("p (h c) -> p h c", c=65)
                    nc.vector.tensor_copy(
                        vdst[:, :, 0:64], pts[tmi][:].rearrange("p (h c) -> p h c", c=64))

        # ---- Phase B: attention + Phase C: out-projection ----
        with tc.tile_pool(name="const", bufs=1) as constp, \
             tc.tile_pool(name="wo", bufs=1) as wop, \
             tc.tile_pool(name="yp", bufs=2) as yp, \
             tc.tile_pool(name="bp", bufs=6) as bp, \
             tc.tile_pool(name="pbp", bufs=16) as pbp, \
             tc.tile_pool(name="co", bufs=2) as cop, \
             tc.tile_pool(name="bps", bufs=2, space="PSUM") as bps, \
             tc.tile_pool(name="yps", bufs=1, space="PSUM") as yps, \
             tc.tile_pool(name="cps", bufs=2, space="PSUM") as cps:
            # diagonal-chunk causal masks: mask[d][i, j] = 0 if j >= i + d*128
            # else -1e30   (scoresT orientation: partition i = key, free j = query)
            masks = []
            for dlt in range(4):
                mt = constp.tile([128, 2, 512], F32, tag=f"m{dlt}", name=f"m{dlt}")
                nc.gpsimd.memset(mt[:], 0.0)
                nc.gpsimd.affine_select(
                    out=mt[:], in_=mt[:],
                    compare_op=mybir.AluOpType.is_ge, fill=-1e30,
                    base=-dlt * 128, pattern=[[0, 2], [1, 512]],
                    channel_multiplier=-1)
                masks.append(mt)
            wo = []
            for ki in range(F // 128):
                t = wop.tile([128, C], F32R, tag=f"wo{ki}", name=f"wo{ki}")
                nc.sync.dma_start(out=t[:], in_=woutt[ki * 128:(ki + 1) * 128, :])
                wo.append(t)

            for qc in range(NT):
                n_kt = qc * 4 + 4
                yts = [yp.tile([128, 512], F32R, tag=f"y{i}", name=f"y{i}")
                       for i in range(4)]
                for hp in range(HPC // 2):       # head pairs (2*hp, 2*hp+1)
                    qpair = qts[hp][:, qc * 512:(qc + 1) * 512]
                    pyA = yps.tile([128, 512], F32, tag="pyA", name="pyA")
                    pyB = yps.tile([128, 512], F32, tag="pyB", name="pyB")
                    pbs = []
                    for kt in range(n_kt):
                        ksl = kts[hp][:, kt * 128:(kt + 1) * 128]
                        # causal trim: cols < lo are fully masked; keep N>=256
                        # (f32r matmul needs moving dim >= 256 for full rate)
                        lo = min(max((kt - qc * 4) * 128, 0), 256)
                        ps = bps.tile([128, 2, 512], F32, tag="ps", name="ps")
                        nc.tensor.matmul(ps[:, 0, lo:512], ksl[0:64, :],
                                         qpair[0:64, lo:512],
                                         start=True, stop=True, tile_position=(0, 0))
                        nc.tensor.matmul(ps[:, 1, lo:512], ksl[64:128, :],
                                         qpair[64:128, lo:512],
                                         start=True, stop=True, tile_position=(64, 0))
                        if kt >= qc * 4:
                            mk = masks[kt - qc * 4]
                            nc.vector.tensor_add(ps[:, :, lo:512], ps[:, :, lo:512],
                                                 mk[:, :, lo:512])
                        pb = pbp.tile([128, 2, 512], BF16, tag="pb", name="pb")
                        nc.scalar.activation(pb[:, :, lo:512], ps[:, :, lo:512],
                                             mybir.ActivationFunctionType.Exp)
                        pbs.append((kt, lo, pb))
                    for kt, lo, pb in pbs:
                        a0 = 2 * hp * 65
                        nc.tensor.matmul(pyA[:, lo:512], vts[kt][:, a0:a0 + 128],
                                         pb[:, 0, lo:512],
                                         start=(kt == 0), stop=(kt == n_kt - 1))
                    for kt, lo, pb in pbs:
                        a0 = 2 * hp * 65 + 65
                        nc.tensor.matmul(pyB[:, lo:512], vts[kt][:, a0:a0 + 128],
                                         pb[:, 1, lo:512],
                                         start=(kt == 0), stop=(kt == n_kt - 1))
                    for hh, py in ((0, pyA), (1, pyB)):
                        h = 2 * hp + hh
                        s1 = bp.tile([1, 512], F32, tag="s1", name="s1")
                        nc.vector.tensor_copy(s1[:], py[64:65, :])
                        r = bp.tile([1, 512], F32, tag="r", name="r")
                        nc.vector.reciprocal_approx_fast(out=r[:], in_=s1[:])
                        rb = bp.tile([64, 512], F32, tag="rb", name="rb")
                        nc.gpsimd.partition_broadcast(rb[:], r[:])
                        half = (h % 2) * 64
                        nc.vector.tensor_mul(yts[h // 2][half:half + 64, :],
                                             py[0:64, :], rb[:])
                # out-projection for this token chunk
                for m in range(8):
                    po = cps.tile([128, 512], F32, tag="po", name="po")
                    for ki in range(F // 128):
                        nc.tensor.matmul(po[:], wo[ki][:, m * 128:(m + 1) * 128],
                                         yts[ki][:],
                                         start=(ki == 0), stop=(ki == F // 128 - 1))
                    oo = cop.tile([128, 512], F32, tag="oo", name="oo")
                    nc.vector.tensor_copy(oo[:], po[:])
                    nc.sync.dma_start(
                        out=out[m * 128:(m + 1) * 128, qc * 512:(qc + 1) * 512],
                        in_=oo[:])
    nc.finalize()
    return nc


def _get_nc():
    global _NC_CACHE
    if _NC_CACHE is None:
        _NC_CACHE = _build()
    return _NC_CACHE


def kernel(x, w_qkv, w_out):
    x = np.ascontiguousarray(np.asarray(x), dtype=np.float32)
    w_qkv = np.asarray(w_qkv, dtype=np.float32)
    w_out = np.asarray(w_out, dtype=np.float32)
    nc = _get_nc()

    in_maps = []
    for c in range(NCORES):
        b, j = divmod(c, 2)
        rows = np.r_[j * F:(j + 1) * F,
                     C + j * F:C + (j + 1) * F,
                     2 * C + j * F:2 * C + (j + 1) * F]
        in_maps.append({
            "xt": np.ascontiguousarray(x[b].T),
            "wqkvt": np.ascontiguousarray(w_qkv[rows, :].T),
            "woutt": np.ascontiguousarray(w_out[:, j * F:(j + 1) * F].T),
        })

    res = run_bass_kernel_spmd(nc, in_maps, core_ids=list(range(NCORES)))
    y = np.empty((B, T, C), np.float32)
    for b in range(B):
        y[b] = (res.results[2 * b]["out"] + res.results[2 * b + 1]["out"]).T
    return y


# revision 11
# speedup vs baseline: 1.0491x; 1.0084x over previous
"""Causal self-attention (B=4, T=2048, C=1024, H=16) on 8 TRN2 NeuronCores.

Sharding: tensor-parallel pairs. Core c handles batch b = c//2 and head-half
j = c%2 (8 of the 16 heads). Each core computes the QKV projection for its
heads, causal attention, and the out-projection contracted over its half of
the features, producing a partial output. The pair-sum (the "all-reduce after
out_proj" of the tensor-parallel scheme) happens at unshard time on the host.

Matmuls run in float32r (TF32-like rounding of fp32 at full PE rate) except
the probs@V matmul, which runs in bf16 with 128-column weight windows (FWL).
Softmax runs in fp32: 1/sqrt(D) folded into the q projection, causal masking
via additive -1e30 before exp, and the denominator comes free from a
ones-column appended to V so the AV matmul accumulates sum(exp) in PSUM.

q/k/v stay resident in SBUF between the projection and attention phases (no
DRAM round-trip). Every core executes the identical program (SPMD); only the
input shards differ.
"""
import numpy as np
from contextlib import ExitStack

import concourse.bass as bass
from concourse import bacc
import concourse.mybir as mybir
import concourse.tile as tile
from concourse.bass_utils import run_bass_kernel_spmd

B, T, C, H, D = 4, 2048, 1024, 16, 64
NCORES = 8
HPC = H // 2          # heads per core
F = HPC * D           # 512 features per core (per q/k/v)
KI = C // 128         # 8 contraction tiles over C
NT = T // 512         # 4 token chunks
F32 = mybir.dt.float32
F32R = mybir.dt.float32r
BF16 = mybir.dt.bfloat16

_NC_CACHE = None


def _build():
    nc = bacc.Bacc("TRN2", target_bir_lowering=False, debug=False)
    # raw fp32 data declared as f32r: the PE rounds on read identically to an
    # explicit rounding pass (verified bit-exact on HW)
    xt = nc.dram_tensor("xt", [C, T], F32R, kind="ExternalInput").ap()
    wqkvt = nc.dram_tensor("wqkvt", [C, 3 * F], F32R, kind="ExternalInput").ap()
    woutt = nc.dram_tensor("woutt", [F, C], F32R, kind="ExternalInput").ap()
    out = nc.dram_tensor("out", [C, T], F32, kind="ExternalOutput").ap()

    with ExitStack() as ctx:
        tc = ctx.enter_context(tile.TileContext(nc))

        # SBUF-resident q/k/v for the whole kernel:
        #   qT[m][128f, T] (pre-scaled by 1/8), kT[m][128f, T] — feature-major
        #   vt[tm][128tk, 583] bf16 — token-major, 8 head-groups of 65 cols
        #   (64 v features + ones col), tail-padded so every 128-col FWL
        #   weight window stays in bounds; pad/ones cols only ever feed
        #   psum partitions >= 65 which are never read.
        qk = ctx.enter_context(tc.tile_pool(name="qk", bufs=1))
        vp = ctx.enter_context(tc.tile_pool(name="vp", bufs=1))
        qts = [qk.tile([128, T], F32R, tag=f"q{m}", name=f"q{m}") for m in range(4)]
        kts = [qk.tile([128, T], F32R, tag=f"k{m}", name=f"k{m}") for m in range(4)]
        vts = [vp.tile([128, 583], BF16, tag=f"v{tm}", name=f"v{tm}")
               for tm in range(T // 128)]

        # ---- Phase A: QKV projection, streamed over 512-token chunks ----
        with tc.tile_pool(name="a_w", bufs=1) as a_w, \
             tc.tile_pool(name="a_x", bufs=3) as a_x, \
             tc.tile_pool(name="a_ps", bufs=2, space="PSUM") as a_ps:
            wq = []
            for ki in range(KI):
                t = a_w.tile([128, 3 * F], F32R, tag=f"w{ki}", name=f"w{ki}")
                wq.append(t)
            for tm in range(T // 128):
                nc.gpsimd.memset(vts[tm][:], 1.0)
            for n in range(NT):
                xc = []
                for ki in range(KI):
                    if n == 0:
                        # interleave weight/input loads in consumption order so
                        # ki-major accumulation starts after the first pair lands
                        nc.sync.dma_start(out=wq[ki][:],
                                          in_=wqkvt[ki * 128:(ki + 1) * 128, :])
                    t = a_x.tile([128, 512], F32R, tag=f"xc{ki}", name=f"xc{ki}")
                    nc.sync.dma_start(
                        out=t[:], in_=xt[ki * 128:(ki + 1) * 128, n * 512:(n + 1) * 512])
                    xc.append(t)
                # q,k feature-major — ki-major over groups of 4 psum banks
                for mg in (0, 4):
                    pts = [a_ps.tile([128, 512], F32, tag=f"pt{i}", name=f"pt{i}")
                           for i in range(4)]
                    for ki in range(KI):
                        for i in range(4):
                            m = mg + i
                            nc.tensor.matmul(pts[i][:], wq[ki][:, m * 128:(m + 1) * 128],
                                             xc[ki][:],
                                             start=(ki == 0), stop=(ki == KI - 1))
                    for i in range(4):
                        m = mg + i
                        dst = (qts[m] if m < 4 else kts[m - 4])[:, n * 512:(n + 1) * 512]
                        if m < 4:
                            nc.vector.tensor_scalar_mul(dst, pts[i][:], 0.125)
                        else:
                            nc.vector.tensor_copy(dst, pts[i][:])
                # v token-major — ki-major over the chunk's 4 token tiles
                pts = [a_ps.tile([128, 512], F32, tag=f"pt{i}", name=f"pt{i}")
                       for i in range(4)]
                for ki in range(KI):
                    for tmi in range(4):
                        nc.tensor.matmul(pts[tmi][:],
                                         xc[ki][:, tmi * 128:(tmi + 1) * 128],
                                         wq[ki][:, 2 * F:3 * F],
                                         start=(ki == 0), stop=(ki == KI - 1))
                for tmi in range(4):
                    tm = n * 4 + tmi
                    vdst = vts[tm][:, 0:520].rearrange("p (h c) -> p h c", c=65)
                    nc.vector.tensor_copy(
                        vdst[:, :, 0:64], pts[tmi][:].rearrange("p (h c) -> p h c", c=64))

        # ---- Phase B: attention + Phase C: out-projection ----
        with tc.tile_pool(name="const", bufs=1) as constp, \
             tc.tile_pool(name="wo", bufs=1) as wop, \
             tc.tile_pool(name="yp", bufs=2) as yp, \
             tc.tile_pool(name="bp", bufs=6) as bp, \
             tc.tile_pool(name="pbp", bufs=16) as pbp, \
             tc.tile_pool(name="co", bufs=2) as cop, \
             tc.tile_pool(name="bps", bufs=2, space="PSUM") as bps, \
             tc.tile_pool(name="yps", bufs=1, space="PSUM") as yps, \
             tc.tile_pool(name="cps", bufs=2, space="PSUM") as cps:
            # diagonal-chunk causal masks: mask[d][i, j] = 0 if j >= i + d*128
            # else -1e30   (scoresT orientation: partition i = key, free j = query)
            masks = []
            for dlt in range(4):
                mt = constp.tile([128, 2, 512], F32, tag=f"m{dlt}", name=f"m{dlt}")
                nc.gpsimd.memset(mt[:], 0.0)
                nc.gpsimd.affine_select(
                    out=mt[:], in_=mt[:],
                    compare_op=mybir.AluOpType.is_ge, fill=-1e30,
                    base=-dlt * 128, pattern=[[0, 2], [1, 512]],
                    channel_multiplier=-1)
                masks.append(mt)
            wo = []
            for ki in range(F // 128):
                t = wop.tile([128, C], F32R, tag=f"wo{ki}", name=f"wo{ki}")
                nc.sync.dma_start(out=t[:], in_=woutt[ki * 128:(ki + 1) * 128, :])
                wo.append(t)

            for qc in range(NT):
                n_kt = qc * 4 + 4
                yts = [yp.tile([128, 512], F32R, tag=f"y{i}", name=f"y{i}")
                       for i in range(4)]
                for hp in range(HPC // 2):       # head pairs (2*hp, 2*hp+1)
                    qpair = qts[hp][:, qc * 512:(qc + 1) * 512]
                    pyA = yps.tile([128, 512], F32, tag="pyA", name="pyA")
                    pyB = yps.tile([128, 512], F32, tag="pyB", name="pyB")
                    pbs = []
                    for kt in range(n_kt):
                        ksl = kts[hp][:, kt * 128:(kt + 1) * 128]
                        # causal trim: cols < lo are fully masked; keep N>=256
                        # (f32r matmul needs moving dim >= 256 for full rate)
                        lo = min(max((kt - qc * 4) * 128, 0), 256)
                        ps = bps.tile([128, 2, 512], F32, tag="ps", name="ps")
                        nc.tensor.matmul(ps[:, 0, lo:512], ksl[0:64, :],
                                         qpair[0:64, lo:512],
                                         start=True, stop=True, tile_position=(0, 0))
                        nc.tensor.matmul(ps[:, 1, lo:512], ksl[64:128, :],
                                         qpair[64:128, lo:512],
                                         start=True, stop=True, tile_position=(64, 0))
                        if kt >= qc * 4:
                            mk = masks[kt - qc * 4]
                            nc.vector.tensor_add(ps[:, :, lo:512], ps[:, :, lo:512],
                                                 mk[:, :, lo:512])
                        pb = pbp.tile([128, 2, 512], BF16, tag="pb", name="pb")
                        nc.scalar.activation(pb[:, :, lo:512], ps[:, :, lo:512],
                                             mybir.ActivationFunctionType.Exp)
                        pbs.append((kt, lo, pb))
                    for kt, lo, pb in pbs:
                        a0 = 2 * hp * 65
                        nc.tensor.matmul(pyA[:, lo:512], vts[kt][:, a0:a0 + 128],
                                         pb[:, 0, lo:512],
                                         start=(kt == 0), stop=(kt == n_kt - 1))
                    for kt, lo, pb in pbs:
                        a0 = 2 * hp * 65 + 65
                        nc.tensor.matmul(pyB[:, lo:512], vts[kt][:, a0:a0 + 128],
                                         pb[:, 1, lo:512],
                                         start=(kt == 0), stop=(kt == n_kt - 1))
                    for hh, py in ((0, pyA), (1, pyB)):
                        h = 2 * hp + hh
                        s1 = bp.tile([1, 512], F32, tag="s1", name="s1")
                        nc.vector.tensor_copy(s1[:], py[64:65, :])
                        r = bp.tile([1, 512], F32, tag="r", name="r")
                        nc.vector.reciprocal_approx_fast(out=r[:], in_=s1[:])
                        rb = bp.tile([64, 512], F32, tag="rb", name="rb")
                        nc.gpsimd.partition_broadcast(rb[:], r[:])
                        half = (h % 2) * 64
                        nc.vector.tensor_mul(yts[h // 2][half:half + 64, :],
                                             py[0:64, :], rb[:])
                # out-projection for this token chunk
                for m in range(8):
                    po = cps.tile([128, 512], F32, tag="po", name="po")
                    for ki in range(F // 128):
                        nc.tensor.matmul(po[:], wo[ki][:, m * 128:(m + 1) * 128],
                                         yts[ki][:],
                                         start=(ki == 0), stop=(ki == F // 128 - 1))
                    oo = cop.tile([128, 512], F32, tag="oo", name="oo")
                    nc.vector.tensor_copy(oo[:], po[:])
                    nc.sync.dma_start(
                        out=out[m * 128:(m + 1) * 128, qc * 512:(qc + 1) * 512],
                        in_=oo[:])
    nc.finalize()
    return nc


def _get_nc():
    global _NC_CACHE
    if _NC_CACHE is None:
        _NC_CACHE = _build()
    return _NC_CACHE


def kernel(x, w_qkv, w_out):
    x = np.ascontiguousarray(np.asarray(x), dtype=np.float32)
    w_qkv = np.asarray(w_qkv, dtype=np.float32)
    w_out = np.asarray(w_out, dtype=np.float32)
    nc = _get_nc()

    in_maps = []
    for c in range(NCORES):
        b, j = divmod(c, 2)
        rows = np.r_[j * F:(j + 1) * F,
                     C + j * F:C + (j + 1) * F,
                     2 * C + j * F:2 * C + (j + 1) * F]
        in_maps.append({
            "xt": np.ascontiguousarray(x[b].T),
            "wqkvt": np.ascontiguousarray(w_qkv[rows, :].T),
            "woutt": np.ascontiguousarray(w_out[:, j * F:(j + 1) * F].T),
        })

    res = run_bass_kernel_spmd(nc, in_maps, core_ids=list(range(NCORES)))
    y = np.empty((B, T, C), np.float32)
    for b in range(B):
        y[b] = (res.results[2 * b]["out"] + res.results[2 * b + 1]["out"]).T
    return y


# revision 12
# speedup vs baseline: 1.0785x; 1.0280x over previous
"""Causal self-attention (B=4, T=2048, C=1024, H=16) on 8 TRN2 NeuronCores.

Sharding: tensor-parallel pairs. Core c handles batch b = c//2 and head-half
j = c%2 (8 of the 16 heads). Each core computes the QKV projection for its
heads, causal attention, and the out-projection contracted over its half of
the features, producing a partial output. The pair-sum (the "all-reduce after
out_proj" of the tensor-parallel scheme) happens at unshard time on the host.

Matmuls run in float32r (TF32-like rounding of fp32 at full PE rate) except
the probs@V matmul, which runs in bf16 with 128-column weight windows (FWL).
Softmax runs in fp32: 1/sqrt(D) folded into the q projection, causal masking
via additive -1e30 before exp, and the denominator comes free from a
ones-column appended to V so the AV matmul accumulates sum(exp) in PSUM.

q/k/v stay resident in SBUF between the projection and attention phases (no
DRAM round-trip). Every core executes the identical program (SPMD); only the
input shards differ.
"""
import ml_dtypes
import numpy as np
from contextlib import ExitStack

import concourse.bass as bass
from concourse import bacc
import concourse.mybir as mybir
import concourse.tile as tile
from concourse.bass_utils import run_bass_kernel_spmd

B, T, C, H, D = 4, 2048, 1024, 16, 64
NCORES = 8
HPC = H // 2          # heads per core
F = HPC * D           # 512 features per core (per q/k/v)
KI = C // 128         # 8 contraction tiles over C
NT = T // 512         # 4 token chunks
F32 = mybir.dt.float32
F32R = mybir.dt.float32r
BF16 = mybir.dt.bfloat16

_NC_CACHE = None


def _build():
    nc = bacc.Bacc("TRN2", target_bir_lowering=False, debug=False)
    # raw fp32 data declared as f32r: the PE rounds on read identically to an
    # explicit rounding pass (verified bit-exact on HW)
    xt = nc.dram_tensor("xt", [C, T], BF16, kind="ExternalInput").ap()
    wqkvt = nc.dram_tensor("wqkvt", [C, 3 * F], BF16, kind="ExternalInput").ap()
    woutt = nc.dram_tensor("woutt", [F, C], F32R, kind="ExternalInput").ap()
    out = nc.dram_tensor("out", [C, T], F32, kind="ExternalOutput").ap()

    with ExitStack() as ctx:
        tc = ctx.enter_context(tile.TileContext(nc))

        # SBUF-resident q/k/v for the whole kernel:
        #   qT[m][128f, T] (pre-scaled by 1/8), kT[m][128f, T] — feature-major
        #   vt[tm][128tk, 583] bf16 — token-major, 8 head-groups of 65 cols
        #   (64 v features + ones col), tail-padded so every 128-col FWL
        #   weight window stays in bounds; pad/ones cols only ever feed
        #   psum partitions >= 65 which are never read.
        qk = ctx.enter_context(tc.tile_pool(name="qk", bufs=1))
        vp = ctx.enter_context(tc.tile_pool(name="vp", bufs=1))
        qts = [qk.tile([128, T], F32R, tag=f"q{m}", name=f"q{m}") for m in range(4)]
        kts = [qk.tile([128, T], F32R, tag=f"k{m}", name=f"k{m}") for m in range(4)]
        vts = [vp.tile([128, 583], BF16, tag=f"v{tm}", name=f"v{tm}")
               for tm in range(T // 128)]

        # ---- Phase A: QKV projection, streamed over 512-token chunks ----
        with tc.tile_pool(name="a_w", bufs=1) as a_w, \
             tc.tile_pool(name="a_x", bufs=3) as a_x, \
             tc.tile_pool(name="a_ps", bufs=2, space="PSUM") as a_ps:
            wq = []
            for ki in range(KI):
                t = a_w.tile([128, 3 * F], BF16, tag=f"w{ki}", name=f"w{ki}")
                wq.append(t)
            for tm in range(T // 128):
                nc.gpsimd.memset(vts[tm][:], 1.0)
            for n in range(NT):
                xc = []
                for ki in range(KI):
                    if n == 0:
                        # interleave weight/input loads in consumption order so
                        # ki-major accumulation starts after the first pair lands
                        nc.sync.dma_start(out=wq[ki][:],
                                          in_=wqkvt[ki * 128:(ki + 1) * 128, :])
                    t = a_x.tile([128, 512], BF16, tag=f"xc{ki}", name=f"xc{ki}")
                    nc.sync.dma_start(
                        out=t[:], in_=xt[ki * 128:(ki + 1) * 128, n * 512:(n + 1) * 512])
                    xc.append(t)
                # q,k feature-major — ki-major over groups of 4 psum banks
                for mg in (0, 4):
                    pts = [a_ps.tile([128, 512], F32, tag=f"pt{i}", name=f"pt{i}")
                           for i in range(4)]
                    for ki in range(KI):
                        for i in range(4):
                            m = mg + i
                            nc.tensor.matmul(pts[i][:], wq[ki][:, m * 128:(m + 1) * 128],
                                             xc[ki][:],
                                             start=(ki == 0), stop=(ki == KI - 1))
                    for i in range(4):
                        m = mg + i
                        dst = (qts[m] if m < 4 else kts[m - 4])[:, n * 512:(n + 1) * 512]
                        if m < 4:
                            nc.vector.tensor_scalar_mul(dst, pts[i][:], 0.125)
                        else:
                            nc.vector.tensor_copy(dst, pts[i][:])
                # v token-major — ki-major over the chunk's 4 token tiles
                pts = [a_ps.tile([128, 512], F32, tag=f"pt{i}", name=f"pt{i}")
                       for i in range(4)]
                for ki in range(KI):
                    for tmi in range(4):
                        nc.tensor.matmul(pts[tmi][:],
                                         xc[ki][:, tmi * 128:(tmi + 1) * 128],
                                         wq[ki][:, 2 * F:3 * F],
                                         start=(ki == 0), stop=(ki == KI - 1))
                for tmi in range(4):
                    tm = n * 4 + tmi
                    vdst = vts[tm][:, 0:520].rearrange("p (h c) -> p h c", c=65)
                    nc.vector.tensor_copy(
                        vdst[:, :, 0:64], pts[tmi][:].rearrange("p (h c) -> p h c", c=64))

        # ---- Phase B: attention + Phase C: out-projection ----
        with tc.tile_pool(name="const", bufs=1) as constp, \
             tc.tile_pool(name="wo", bufs=1) as wop, \
             tc.tile_pool(name="yp", bufs=2) as yp, \
             tc.tile_pool(name="bp", bufs=6) as bp, \
             tc.tile_pool(name="pbp", bufs=16) as pbp, \
             tc.tile_pool(name="co", bufs=2) as cop, \
             tc.tile_pool(name="bps", bufs=2, space="PSUM") as bps, \
             tc.tile_pool(name="yps", bufs=1, space="PSUM") as yps, \
             tc.tile_pool(name="cps", bufs=2, space="PSUM") as cps:
            # diagonal-chunk causal masks: mask[d][i, j] = 0 if j >= i + d*128
            # else -1e30   (scoresT orientation: partition i = key, free j = query)
            masks = []
            for dlt in range(4):
                mt = constp.tile([128, 2, 512], F32, tag=f"m{dlt}", name=f"m{dlt}")
                nc.gpsimd.memset(mt[:], 0.0)
                nc.gpsimd.affine_select(
                    out=mt[:], in_=mt[:],
                    compare_op=mybir.AluOpType.is_ge, fill=-1e30,
                    base=-dlt * 128, pattern=[[0, 2], [1, 512]],
                    channel_multiplier=-1)
                masks.append(mt)
            wo = []
            for ki in range(F // 128):
                t = wop.tile([128, C], F32R, tag=f"wo{ki}", name=f"wo{ki}")
                nc.sync.dma_start(out=t[:], in_=woutt[ki * 128:(ki + 1) * 128, :])
                wo.append(t)

            for qc in range(NT):
                n_kt = qc * 4 + 4
                yts = [yp.tile([128, 512], F32R, tag=f"y{i}", name=f"y{i}")
                       for i in range(4)]
                for hp in range(HPC // 2):       # head pairs (2*hp, 2*hp+1)
                    qpair = qts[hp][:, qc * 512:(qc + 1) * 512]
                    pyA = yps.tile([128, 512], F32, tag="pyA", name="pyA")
                    pyB = yps.tile([128, 512], F32, tag="pyB", name="pyB")
                    pbs = []
                    for kt in range(n_kt):
                        ksl = kts[hp][:, kt * 128:(kt + 1) * 128]
                        # causal trim: cols < lo are fully masked; keep N>=256
                        # (f32r matmul needs moving dim >= 256 for full rate)
                        lo = min(max((kt - qc * 4) * 128, 0), 256)
                        ps = bps.tile([128, 2, 512], F32, tag="ps", name="ps")
                        nc.tensor.matmul(ps[:, 0, lo:512], ksl[0:64, :],
                                         qpair[0:64, lo:512],
                                         start=True, stop=True, tile_position=(0, 0))
                        nc.tensor.matmul(ps[:, 1, lo:512], ksl[64:128, :],
                                         qpair[64:128, lo:512],
                                         start=True, stop=True, tile_position=(64, 0))
                        if kt >= qc * 4:
                            mk = masks[kt - qc * 4]
                            nc.vector.tensor_add(ps[:, :, lo:512], ps[:, :, lo:512],
                                                 mk[:, :, lo:512])
                        pb = pbp.tile([128, 2, 512], BF16, tag="pb", name="pb")
                        nc.scalar.activation(pb[:, :, lo:512], ps[:, :, lo:512],
                                             mybir.ActivationFunctionType.Exp)
                        pbs.append((kt, lo, pb))
                    for kt, lo, pb in pbs:
                        a0 = 2 * hp * 65
                        nc.tensor.matmul(pyA[:, lo:512], vts[kt][:, a0:a0 + 128],
                                         pb[:, 0, lo:512],
                                         start=(kt == 0), stop=(kt == n_kt - 1))
                    for kt, lo, pb in pbs:
                        a0 = 2 * hp * 65 + 65
                        nc.tensor.matmul(pyB[:, lo:512], vts[kt][:, a0:a0 + 128],
                                         pb[:, 1, lo:512],
                                         start=(kt == 0), stop=(kt == n_kt - 1))
                    for hh, py in ((0, pyA), (1, pyB)):
                        h = 2 * hp + hh
                        s1 = bp.tile([1, 512], F32, tag="s1", name="s1")
                        nc.vector.tensor_copy(s1[:], py[64:65, :])
                        r = bp.tile([1, 512], F32, tag="r", name="r")
                        nc.vector.reciprocal_approx_fast(out=r[:], in_=s1[:])
                        rb = bp.tile([64, 512], F32, tag="rb", name="rb")
                        nc.gpsimd.partition_broadcast(rb[:], r[:])
                        half = (h % 2) * 64
                        nc.vector.tensor_mul(yts[h // 2][half:half + 64, :],
                                             py[0:64, :], rb[:])
                # out-projection for this token chunk
                for m in range(8):
                    po = cps.tile([128, 512], F32, tag="po", name="po")
                    for ki in range(F // 128):
                        nc.tensor.matmul(po[:], wo[ki][:, m * 128:(m + 1) * 128],
                                         yts[ki][:],
                                         start=(ki == 0), stop=(ki == F // 128 - 1))
                    oo = cop.tile([128, 512], F32, tag="oo", name="oo")
                    nc.vector.tensor_copy(oo[:], po[:])
                    nc.sync.dma_start(
                        out=out[m * 128:(m + 1) * 128, qc * 512:(qc + 1) * 512],
                        in_=oo[:])
    nc.finalize()
    return nc


def _get_nc():
    global _NC_CACHE
    if _NC_CACHE is None:
        _NC_CACHE = _build()
    return _NC_CACHE


def kernel(x, w_qkv, w_out):
    x = np.ascontiguousarray(np.asarray(x), dtype=np.float32)
    w_qkv = np.asarray(w_qkv, dtype=np.float32)
    w_out = np.asarray(w_out, dtype=np.float32)
    nc = _get_nc()

    in_maps = []
    for c in range(NCORES):
        b, j = divmod(c, 2)
        rows = np.r_[j * F:(j + 1) * F,
                     C + j * F:C + (j + 1) * F,
                     2 * C + j * F:2 * C + (j + 1) * F]
        in_maps.append({
            "xt": np.ascontiguousarray(x[b].T).astype(ml_dtypes.bfloat16),
            "wqkvt": np.ascontiguousarray(w_qkv[rows, :].T).astype(ml_dtypes.bfloat16),
            "woutt": np.ascontiguousarray(w_out[:, j * F:(j + 1) * F].T),
        })

    res = run_bass_kernel_spmd(nc, in_maps, core_ids=list(range(NCORES)))
    y = np.empty((B, T, C), np.float32)
    for b in range(B):
        y[b] = (res.results[2 * b]["out"] + res.results[2 * b + 1]["out"]).T
    return y
